# revision 20
# baseline (speedup 1.0000x reference)
"""MoE layer (SwiGLU experts, top-2 routing) on 8 Trainium2 NeuronCores.

Strategy (expert parallelism, per the sharding hint):
  - The router (a [N,8] matmul + softmax + top-2, ~0.01% of total FLOPs) is
    computed host-side in float64; it determines the token->expert dispatch.
  - Token dispatch/combine (the "all-to-all") is done host-side: each core e
    receives expert e's weights plus the tokens routed to expert e, padded to
    a uniform capacity C (multiple of 128, same on all cores for SPMD).
  - Each core runs the heavy compute: y = (silu(x@wg) * (x@wu)) @ wd scaled
    by the per-token combine weight. All matmul operands are bf16 (1 cyc/row
    on the PE, same rate as fp32r, but half the HBM/SBUF footprint and FWL
    weight loads); PSUM accumulation stays fp32, well within 2e-2 tolerance.
  - Host scatter-adds each expert's output rows back into the final output.
  - Weights are pre-tiled host-side so every DMA reads >=2KB contiguous per
    partition line; wd (8.4 MB in bf16) is loaded once into SBUF and stays
    resident for all of stage 2.

Device kernel structure (per core):
  Tokens are processed in groups of up to 768 (first group smallest so the
  first matmul's DMA window is short). Stage 1 computes
  hT[f, token] = silu(wg.T x) * (wu.T x) for all F=4096 rows of the group,
  accumulating over D=1024 in PSUM (8 matmuls per 128-row f-tile), with the
  gate/up PSUM banks drained by ScalarE (silu) and VectorE (mul, bf16 out)
  into SBUF. Stage 2 contracts hT over F entirely in PSUM (32-matmul
  accumulation per output tile), applies the combine weight, and streams
  results out. The very last PSUM batch is a single c-tile so the final
  drain exposes only ~1us.
"""

import os
import sys

sys.path.insert(0, "/opt/trn_rl_repo")
import numpy as np
import ml_dtypes

BF16 = ml_dtypes.bfloat16

P = 128
D_MODEL = 1024
D_FF = 4096
N_EXPERTS = 8
TOP_K = 2
G_MAX = 768  # token group size: hT for the group stays in SBUF
N_WARMUP = 28  # PE warmup matmuls: ramp the clock + bridge the first DMAs
N_PRE_FT = 3  # f-tiles of next group's wg/wu prefetched during stage 1

LAST_EXEC_NS = None
_programs = {}


def _ensure_axon_hooks():
    """The agent image's antenv lacks axon_hooks; reconstruct it so
    trace=True works (NTFF profiling via libaxon_pjrt ctypes hook)."""
    import types

    try:
        import antenv.axon_hooks  # noqa: F401

        return
    except ImportError:
        pass
    try:
        import antenv

        mod = types.ModuleType("antenv.axon_hooks")
        _hook = [None]
        mod.set_axon_ntff_profile_hook = lambda h: _hook.__setitem__(0, h)
        mod.get_axon_ntff_profile_hook = lambda: _hook[0]
        sys.modules["antenv.axon_hooks"] = mod
        antenv.axon_hooks = mod
        if "/root/.axon_site" not in sys.path:
            sys.path.insert(0, "/root/.axon_site")
        from trn_agent_boot.trn_boot import _ntff_profile_via_ctypes

        mod.set_axon_ntff_profile_hook(
            _ntff_profile_via_ctypes("/opt/axon/libaxon_pjrt.so")
        )
        import concourse.bass_utils as bu

        bu.upload_artifacts = lambda tmpdir: f"local://{tmpdir}"
    except Exception:
        pass


def _chunks(gc):
    n_ch = -(-gc // 512)
    base_w, extra = divmod(gc, n_ch)
    out, c0 = [], 0
    for ci in range(n_ch):
        cw = base_w + (1 if ci < extra else 0)
        out.append(cw)
        c0 += cw
    return out


def _group_sizes(C):
    sizes = []
    rem = C
    while rem > 0:
        if rem >= G_MAX + 512 or rem <= G_MAX:
            take = min(G_MAX, rem)
        else:
            take = rem - 512
        sizes.append(take)
        rem -= take
    sizes.sort()
    return sizes


def _build_program(C):
    import concourse.bacc as bacc
    import concourse.mybir as mybir
    from concourse.tile import TileContext

    fp32 = mybir.dt.float32
    bf16 = mybir.dt.bfloat16
    D, F = D_MODEL, D_FF
    DT, FT = D // P, F // P
    GS = C // P  # total 128-token tiles
    silu_fn = mybir.ActivationFunctionType.Silu
    mult_op = mybir.AluOpType.mult

    nc = bacc.Bacc(
        "TRN2", target_bir_lowering=False, debug=False, num_devices=N_EXPERTS
    )
    # host pre-tiled layouts (see kernel()); startup is DMA-completion-
    # latency bound, so gate+up share one tensor (one DMA per f-tile) and the
    # token slab is a single DMA:
    #   xT[p, :]                 = group-major flat token slabs: for each
    #                              group, [dt, c] row-major (one contiguous
    #                              10-12KB run per partition per group)
    #   wgu[ft, p, 0/1, dt, j]   = w_gate/w_up[dt*128+p, ft*128+j] (gate half
    #                              contiguous first: the first matmul chain
    #                              needs only 0.25 MB)
    #   wdT[p, ft, d]            = w_down[ft*128+p, d]   (SBUF-resident)
    #   scT[p, g]                = combine_weight[g*128+p]
    xT = nc.dram_tensor("xT", [P, DT * C], bf16, kind="ExternalInput")
    wgu_t = nc.dram_tensor("wgu", [FT, P, 2, DT, P], bf16, kind="ExternalInput")
    wd = nc.dram_tensor("wd", [P, FT, D], bf16, kind="ExternalInput")
    sc = nc.dram_tensor("sc", [P, GS], fp32, kind="ExternalInput")
    # group-0 starter: [gate0+up0 | xg chunk1 | wgu ft1..npre-1] packed so the
    # whole critical set lands as 128 fat descriptors (early DMA is
    # descriptor-rate bound at ~50/us)
    _npre = min(N_PRE_FT, FT)
    _cw1 = _chunks(_group_sizes(C)[0])[0]
    ST_LEN = 2 * DT * P + DT * _cw1 + (_npre - 1) * 2 * DT * P
    st = nc.dram_tensor("st", [P, ST_LEN], bf16, kind="ExternalInput")
    y = nc.dram_tensor("y", [C, D], bf16, kind="ExternalOutput")

    xT_ap = xT.ap()
    st_ap = st.ap()
    wgu_ap = wgu_t.ap()
    wd_ap = wd.ap()
    sc_ap = sc.ap()
    y_ap = y.ap()

    # groups of <=768, avoiding tails <512; smallest first so the initial
    # DMA window is short
    sizes = _group_sizes(C)
    groups = []
    g0 = 0
    for gc in sizes:
        groups.append((g0, gc))
        g0 += gc
    n_groups = len(groups)

    def batch_plan(gsub, peel_tail):
        # one batch if it fits the 6 stage-2 PSUM banks, else split evenly.
        # peel_tail: split a single final c-tile into its own batch so the
        # terminal PSUM drain is one eviction, not gsub of them.
        if peel_tail and gsub > 1:
            head = batch_plan(gsub - 1, False)
            return head + [[gsub - 1]]
        if gsub <= 6:
            return [list(range(gsub))]
        n_b = (gsub + 5) // 6
        base, extra = divmod(gsub, n_b)
        out, s = [], 0
        for i in range(n_b):
            n = base + (1 if i < extra else 0)
            out.append(list(range(s, s + n)))
            s += n
        return out

    with TileContext(nc) as tc:
        with (
            tc.tile_pool(name="warm", bufs=1) as warm_pool,
            tc.tile_pool(name="xg", bufs=4) as xg_pool,
            tc.tile_pool(name="wgu", bufs=3) as wgu_pool,
            tc.tile_pool(name="wgp", bufs=1) as wgu_pre_pool,
            tc.tile_pool(name="ht", bufs=FT) as ht_pool,
            tc.tile_pool(name="wdr", bufs=1) as wd_pool,
            tc.tile_pool(name="act", bufs=2) as act_pool,
            tc.tile_pool(name="out", bufs=2) as out_pool,
            tc.tile_pool(name="scp", bufs=2) as sc_pool,
            tc.tile_pool(name="ps1", bufs=1, space="PSUM") as ps1_pool,
            tc.tile_pool(name="ps2", bufs=6, space="PSUM") as ps2_pool,
        ):
            # Warm-up: keep TensorE busy while the first tiles stream in, so
            # the HAM clock gate reaches full speed before real matmuls start.
            wsrc = warm_pool.tile([P, 256], bf16, name="wsrc")
            nc.vector.memset(wsrc[:], 0.0)
            wps = ps1_pool.tile([P, 512], fp32, name="psg")
            for wi in range(N_WARMUP):
                nc.tensor.matmul(
                    wps[:, :256],
                    wsrc[:, :P],
                    wsrc[:],
                    start=(wi == 0),
                    stop=(wi == N_WARMUP - 1),
                )

            # prefetch state: (wgt/wut tiles by ft, xg, sct) per group
            pre = {}

            def issue_group_loads(gi):
                g0, gc = groups[gi]
                gsub = gc // P
                npre = min(N_PRE_FT, FT)
                cws = _chunks(gc)
                if gi == 0:
                    # the packed starter: two fat DMAs on parallel queues
                    # carry gate0+up0+chunk1 and wgu ft1..npre-1
                    cw1 = cws[0]
                    o_c1 = 2 * DT * P
                    o_w1 = o_c1 + DT * cw1
                    stt = wgu_pre_pool.tile([P, ST_LEN], bf16, name="wgup")
                    nc.gpsimd.dma_start(out=stt[:, :o_w1], in_=st_ap[:, :o_w1])
                    if ST_LEN > o_w1:
                        nc.sync.dma_start(
                            out=stt[:, o_w1:], in_=st_ap[:, o_w1:]
                        )
                    wgu = {
                        0: stt[:, :o_c1].rearrange(
                            "p (t d j) -> p t d j", t=2, d=DT
                        )
                    }
                    for f in range(1, npre):
                        o = o_w1 + (f - 1) * 2 * DT * P
                        wgu[f] = stt[:, o : o + 2 * DT * P].rearrange(
                            "p (t d j) -> p t d j", t=2, d=DT
                        )
                    xg = [
                        stt[:, o_c1:o_w1].rearrange("p (d c) -> p d c", d=DT)
                    ]
                    off = DT * (g0 + cw1)
                    rest = cws[1:]
                else:
                    # critical-path order: first f-tile's weights (sync)
                    # race the token chunks (gpsimd)
                    block = wgu_pre_pool.tile(
                        [P, npre, 2, DT, P], bf16, name="wgup"
                    )
                    nc.sync.dma_start(out=block[:, 0], in_=wgu_ap[0])
                    wgu = {ft: block[:, ft] for ft in range(npre)}
                    xg = []
                    off = DT * g0
                    rest = cws
                for cw in rest:
                    xt = xg_pool.tile([P, DT, cw], bf16, name="xg")
                    nc.gpsimd.dma_start(
                        out=xt[:],
                        in_=xT_ap[:, off : off + DT * cw].rearrange(
                            "p (dt c) -> p dt c", c=cw
                        ),
                    )
                    xg.append(xt)
                    off += DT * cw
                sct = sc_pool.tile([P, GS], fp32, name="sct")
                nc.gpsimd.dma_start(
                    out=sct[:, :gsub], in_=sc_ap[:, g0 // P : g0 // P + gsub]
                )
                if gi != 0:
                    for ft in range(1, npre):
                        nc.sync.dma_start(out=block[:, ft], in_=wgu_ap[ft])
                pre[gi] = (wgu, xg, sct)

            issue_group_loads(0)
            wd_res = wd_pool.tile([P, FT, D], bf16, name="wdr")

            for gi, (g0, gc) in enumerate(groups):
                gsub = gc // P
                batches = batch_plan(gsub, False)
                batches_last_d0 = batch_plan(gsub, gi == n_groups - 1)

                wgu_pre, xg, sct = pre.pop(gi)

                # equal-width chunks: keep every chunk >=320 so the fixed
                # per-matmul cost stays hidden; chunk ci lives in tile xg[ci]
                chunks = []
                c0 = 0
                for ci, cw in enumerate(_chunks(gc)):
                    chunks.append((ci, c0, cw))
                    c0 += cw

                # ---- stage 1: hT[f, c] = silu(wg.T x) * (wu.T x) ----
                ht_tiles = []
                for ft in range(FT):
                    if ft in wgu_pre:
                        wgut = wgu_pre.pop(ft)
                    else:
                        wgut = wgu_pool.tile([P, 2, DT, P], bf16, name="wgut")
                        nc.sync.dma_start(out=wgut[:], in_=wgu_ap[ft])
                    if gi == 0 and 1 <= ft <= 4:
                        # wd is small in bf16 (8.4 MB): stream it once into
                        # SBUF on the otherwise-idle scalar queue (in
                        # quarters, so stage 2's first f-tiles don't wait on
                        # the whole transfer); never touches HBM again
                        q8 = FT // 4
                        f0 = (ft - 1) * q8
                        nc.scalar.dma_start(
                            out=wd_res[:, f0 : f0 + q8, :],
                            in_=wd_ap[:, f0 : f0 + q8, :],
                        )
                    if ft == (24 if gc >= 640 else 12) and gi + 1 < n_groups:
                        # queue the next group's token slab + first weights
                        # behind the remaining stage-1 loads: they land
                        # during this group's stage 2
                        issue_group_loads(gi + 1)
                    ht = ht_pool.tile([P, G_MAX], bf16, name="ht")
                    ht_tiles.append(ht)
                    for ci, c0, cw in chunks:
                        psg = ps1_pool.tile([P, 512], fp32, name="psg")
                        for dt_i in range(DT):
                            nc.tensor.matmul(
                                psg[:, :cw],
                                wgut[:, 0, dt_i, :],
                                xg[ci][:, dt_i, :],
                                start=(dt_i == 0),
                                stop=(dt_i == DT - 1),
                            )
                        psu = ps1_pool.tile([P, 512], fp32, name="psu")
                        for dt_i in range(DT):
                            nc.tensor.matmul(
                                psu[:, :cw],
                                wgut[:, 1, dt_i, :],
                                xg[ci][:, dt_i, :],
                                start=(dt_i == 0),
                                stop=(dt_i == DT - 1),
                            )
                        sil = act_pool.tile([P, 512], fp32, name="sil")
                        nc.scalar.activation(sil[:, :cw], psg[:, :cw], silu_fn)
                        nc.vector.tensor_tensor(
                            out=ht[:, c0 : c0 + cw],
                            in0=sil[:, :cw],
                            in1=psu[:, :cw],
                            op=mult_op,
                        )

                # ---- stage 2: y[c, d] = sum_f hT[f, c] * wd[f, d], scaled ----
                for d0 in range(0, D_MODEL, 512):
                    cur_batches = batches_last_d0 if d0 == 512 else batches
                    for bi, cs_list in enumerate(cur_batches):
                        ps_out = [
                            ps2_pool.tile([P, 512], fp32, name="pso") for _ in cs_list
                        ]
                        for ft in range(FT):
                            for i, cs in enumerate(cs_list):
                                nc.tensor.matmul(
                                    ps_out[i][:],
                                    ht_tiles[ft][:, cs * P : (cs + 1) * P],
                                    wd_res[:, ft, d0 : d0 + 512],
                                    start=(ft == 0),
                                    stop=(ft == FT - 1),
                                )
                        for i, cs in enumerate(cs_list):
                            ot = out_pool.tile([P, 512], bf16, name="ot")
                            if i % 2 == 0:
                                nc.vector.tensor_scalar_mul(
                                    ot[:], ps_out[i][:], sct[:, cs : cs + 1]
                                )
                            else:
                                # spread evictions across engines so the bank
                                # ring frees faster at d0 boundaries
                                nc.scalar.activation(
                                    ot[:],
                                    ps_out[i][:],
                                    mybir.ActivationFunctionType.Copy,
                                    scale=sct[:, cs : cs + 1],
                                )
                            r0 = g0 + cs * P
                            nc.gpsimd.dma_start(
                                out=y_ap[r0 : r0 + P, d0 : d0 + 512], in_=ot[:]
                            )
    nc.compile()
    return nc


def _get_program(C):
    if C not in _programs:
        _programs[C] = _build_program(C)
    return _programs[C]


def _route(xf, router_w):
    """Host router, float64 (all f32 evaluation orders agree on this input's
    top-2 sets; f64 is the stable reference ranking). Mirrors
    softmax -> top_k(2) -> renormalize from the reference."""
    logits = xf.astype(np.float64) @ router_w.astype(np.float64).T
    logits -= logits.max(axis=-1, keepdims=True)
    sm = np.exp(logits)
    sm /= sm.sum(axis=-1, keepdims=True)
    top = np.argsort(-sm, axis=-1, kind="stable")[:, :TOP_K]
    tsc = np.take_along_axis(sm, top, axis=1)
    tsc = tsc / tsc.sum(axis=-1, keepdims=True)
    return top, tsc


def kernel(x, router_w, w_gate, w_up, w_down):
    global LAST_EXEC_NS
    from concourse.bass_utils import run_bass_kernel_spmd

    trace = os.environ.get("MOE_TRACE", "0") == "1"
    if trace:
        _ensure_axon_hooks()

    x = np.asarray(x, dtype=np.float32)
    router_w = np.asarray(router_w, dtype=np.float32)
    w_gate = np.asarray(w_gate, dtype=np.float32)
    w_up = np.asarray(w_up, dtype=np.float32)
    w_down = np.asarray(w_down, dtype=np.float32)

    B, T, D = x.shape
    N = B * T
    F = D_FF
    FT, DT = F // P, D // P
    xf = np.ascontiguousarray(x.reshape(N, D))

    top, tsc = _route(xf, router_w)

    tok_rows = []
    tok_wts = []
    for e in range(N_EXPERTS):
        mask = top == e
        rows = np.nonzero(mask.any(axis=1))[0]
        wts = tsc[mask].astype(np.float32)
        tok_rows.append(rows)
        tok_wts.append(wts)

    cmax = max(max(len(r) for r in tok_rows), 1)
    C = max(((cmax + P - 1) // P) * P, 256)

    nc = _get_program(C)

    # pre-tile weights host-side (bf16) so device DMAs are contiguous and
    # gate+up arrive in one DMA per f-tile:
    #   wguT[e][ft, p, 0/1, dt, j] = w_gate/w_up[e, dt*128+p, ft*128+j]
    #   wdT[e][p, ft, d]           = w_down[e, ft*128+p, d]
    wg16 = w_gate.astype(BF16).reshape(N_EXPERTS, DT, P, FT, P).transpose(0, 3, 2, 1, 4)
    wu16 = w_up.astype(BF16).reshape(N_EXPERTS, DT, P, FT, P).transpose(0, 3, 2, 1, 4)
    wgu16 = np.ascontiguousarray(np.stack([wg16, wu16], axis=3))
    wd16 = np.ascontiguousarray(
        w_down.astype(BF16).reshape(N_EXPERTS, FT, P, D).transpose(0, 2, 1, 3)
    )

    sizes = _group_sizes(C)
    in_maps = []
    for e in range(N_EXPERTS):
        rows = tok_rows[e]
        xg = np.zeros((C, D), np.float32)
        xg[: len(rows)] = xf[rows]
        # group-major flat layout: per group a contiguous [dt, c] slab per
        # partition row
        x16 = xg.astype(BF16).reshape(C, DT, P)
        parts = []
        g0 = 0
        for gc in sizes:
            for cw in _chunks(gc):
                parts.append(
                    x16[g0 : g0 + cw].transpose(2, 1, 0).reshape(P, DT * cw)
                )
                g0 += cw
        xflat = np.ascontiguousarray(np.concatenate(parts, axis=1))
        npre = min(N_PRE_FT, FT)
        st_parts = [wgu16[e][0].reshape(P, 2 * DT * P), parts[0]]
        for f in range(1, npre):
            st_parts.append(wgu16[e][f].reshape(P, 2 * DT * P))
        stflat = np.ascontiguousarray(np.concatenate(st_parts, axis=1))
        scv = np.zeros((C,), np.float32)
        scv[: len(rows)] = tok_wts[e]
        in_maps.append(
            {
                "xT": xflat,
                "st": stflat,
                "wgu": wgu16[e],
                "wd": wd16[e],
                "sc": np.ascontiguousarray(scv.reshape(C // P, P).T),
            }
        )

    res = run_bass_kernel_spmd(nc, in_maps, list(range(N_EXPERTS)), trace=trace)
    if trace:
        LAST_EXEC_NS = res.exec_time_ns

    out = np.zeros((N, D), np.float32)
    for e in range(N_EXPERTS):
        rows = tok_rows[e]
        out[rows] += res.results[e]["y"][: len(rows)].astype(np.float32)
    return out.reshape(B, T, D)


# revision 22
# speedup vs baseline: 1.0103x; 1.0103x over previous
"""MoE layer (SwiGLU experts, top-2 routing) on 8 Trainium2 NeuronCores.

Strategy (expert parallelism, per the sharding hint):
  - The router (a [N,8] matmul + softmax + top-2, ~0.01% of total FLOPs) is
    computed host-side in float64; it determines the token->expert dispatch.
  - Token dispatch/combine (the "all-to-all") is done host-side: each core e
    receives expert e's weights plus the tokens routed to expert e, padded to
    a uniform capacity C (multiple of 128, same on all cores for SPMD).
  - Each core runs the heavy compute: y = (silu(x@wg) * (x@wu)) @ wd scaled
    by the per-token combine weight. All matmul operands are bf16 (1 cyc/row
    on the PE, same rate as fp32r, but half the HBM/SBUF footprint and FWL
    weight loads); PSUM accumulation stays fp32, well within 2e-2 tolerance.
  - Host scatter-adds each expert's output rows back into the final output.
  - Weights are pre-tiled host-side so every DMA reads >=2KB contiguous per
    partition line; wd (8.4 MB in bf16) is loaded once into SBUF and stays
    resident for all of stage 2.

Device kernel structure (per core):
  Tokens are processed in groups of up to 768 (first group smallest so the
  first matmul's DMA window is short). Stage 1 computes
  hT[f, token] = silu(wg.T x) * (wu.T x) for all F=4096 rows of the group,
  accumulating over D=1024 in PSUM (8 matmuls per 128-row f-tile), with the
  gate/up PSUM banks drained by ScalarE (silu) and VectorE (mul, bf16 out)
  into SBUF. Stage 2 contracts hT over F entirely in PSUM (32-matmul
  accumulation per output tile), applies the combine weight, and streams
  results out. The very last PSUM batch is a single c-tile so the final
  drain exposes only ~1us.
"""

import os
import sys

sys.path.insert(0, "/opt/trn_rl_repo")
import numpy as np
import ml_dtypes

BF16 = ml_dtypes.bfloat16

P = 128
D_MODEL = 1024
D_FF = 4096
N_EXPERTS = 8
TOP_K = 2
G_MAX = 768  # token group size: hT for the group stays in SBUF
N_WARMUP = 100  # PE warmup matmuls: ramp the clock + bridge the first DMAs
N_PRE_FT = 4  # f-tiles of next group's wg/wu prefetched during stage 1

LAST_EXEC_NS = None
_programs = {}


def _ensure_axon_hooks():
    """The agent image's antenv lacks axon_hooks; reconstruct it so
    trace=True works (NTFF profiling via libaxon_pjrt ctypes hook)."""
    import types

    try:
        import antenv.axon_hooks  # noqa: F401

        return
    except ImportError:
        pass
    try:
        import antenv

        mod = types.ModuleType("antenv.axon_hooks")
        _hook = [None]
        mod.set_axon_ntff_profile_hook = lambda h: _hook.__setitem__(0, h)
        mod.get_axon_ntff_profile_hook = lambda: _hook[0]
        sys.modules["antenv.axon_hooks"] = mod
        antenv.axon_hooks = mod
        if "/root/.axon_site" not in sys.path:
            sys.path.insert(0, "/root/.axon_site")
        from trn_agent_boot.trn_boot import _ntff_profile_via_ctypes

        mod.set_axon_ntff_profile_hook(
            _ntff_profile_via_ctypes("/opt/axon/libaxon_pjrt.so")
        )
        import concourse.bass_utils as bu

        bu.upload_artifacts = lambda tmpdir: f"local://{tmpdir}"
    except Exception:
        pass


def _chunks(gc):
    n_ch = -(-gc // 512)
    base_w, extra = divmod(gc, n_ch)
    out, c0 = [], 0
    for ci in range(n_ch):
        cw = base_w + (1 if ci < extra else 0)
        out.append(cw)
        c0 += cw
    return out


def _group_sizes(C):
    sizes = []
    rem = C
    while rem > 0:
        if rem >= G_MAX + 512 or rem <= G_MAX:
            take = min(G_MAX, rem)
        else:
            take = rem - 512
        sizes.append(take)
        rem -= take
    sizes.sort()
    return sizes


def _build_program(C):
    import concourse.bacc as bacc
    import concourse.mybir as mybir
    from concourse.tile import TileContext

    fp32 = mybir.dt.float32
    bf16 = mybir.dt.bfloat16
    D, F = D_MODEL, D_FF
    DT, FT = D // P, F // P
    GS = C // P  # total 128-token tiles
    silu_fn = mybir.ActivationFunctionType.Silu
    mult_op = mybir.AluOpType.mult

    nc = bacc.Bacc(
        "TRN2", target_bir_lowering=False, debug=False, num_devices=N_EXPERTS
    )
    # host pre-tiled layouts (see kernel()); startup is DMA-completion-
    # latency bound, so gate+up share one tensor (one DMA per f-tile) and the
    # token slab is a single DMA:
    #   xT[p, :]                 = group-major flat token slabs: for each
    #                              group, [dt, c] row-major (one contiguous
    #                              10-12KB run per partition per group)
    #   wgu[ft, p, 0/1, dt, j]   = w_gate/w_up[dt*128+p, ft*128+j] (gate half
    #                              contiguous first: the first matmul chain
    #                              needs only 0.25 MB)
    #   wdT[p, ft, d]            = w_down[ft*128+p, d]   (SBUF-resident)
    #   scT[p, g]                = combine_weight[g*128+p]
    xT = nc.dram_tensor("xT", [P, DT * C], bf16, kind="ExternalInput")
    wgu_t = nc.dram_tensor("wgu", [FT, P, 2, DT, P], bf16, kind="ExternalInput")
    wd = nc.dram_tensor("wd", [P, FT, D], bf16, kind="ExternalInput")
    sc = nc.dram_tensor("sc", [P, GS], fp32, kind="ExternalInput")
    y = nc.dram_tensor("y", [C, D], bf16, kind="ExternalOutput")

    xT_ap = xT.ap()
    wgu_ap = wgu_t.ap()
    wd_ap = wd.ap()
    sc_ap = sc.ap()
    y_ap = y.ap()

    # groups of <=768, avoiding tails <512; smallest first so the initial
    # DMA window is short
    sizes = _group_sizes(C)
    groups = []
    g0 = 0
    for gc in sizes:
        groups.append((g0, gc))
        g0 += gc
    n_groups = len(groups)

    def batch_plan(gsub, peel_tail):
        # one batch if it fits the 6 stage-2 PSUM banks, else split evenly.
        # peel_tail: split a single final c-tile into its own batch so the
        # terminal PSUM drain is one eviction, not gsub of them.
        if peel_tail and gsub > 1:
            head = batch_plan(gsub - 1, False)
            return head + [[gsub - 1]]
        if gsub <= 6:
            return [list(range(gsub))]
        n_b = (gsub + 5) // 6
        base, extra = divmod(gsub, n_b)
        out, s = [], 0
        for i in range(n_b):
            n = base + (1 if i < extra else 0)
            out.append(list(range(s, s + n)))
            s += n
        return out

    with TileContext(nc) as tc:
        with (
            tc.tile_pool(name="warm", bufs=1) as warm_pool,
            tc.tile_pool(name="xg", bufs=4) as xg_pool,
            tc.tile_pool(name="wgu", bufs=3) as wgu_pool,
            tc.tile_pool(name="wgp", bufs=1) as wgu_pre_pool,
            tc.tile_pool(name="ht", bufs=FT) as ht_pool,
            tc.tile_pool(name="wdr", bufs=1) as wd_pool,
            tc.tile_pool(name="act", bufs=2) as act_pool,
            tc.tile_pool(name="out", bufs=2) as out_pool,
            tc.tile_pool(name="scp", bufs=2) as sc_pool,
            tc.tile_pool(name="ps1", bufs=1, space="PSUM") as ps1_pool,
            tc.tile_pool(name="ps2", bufs=6, space="PSUM") as ps2_pool,
        ):
            # Warm-up: keep TensorE busy while the first tiles stream in, so
            # the HAM clock gate reaches full speed before real matmuls start.
            wsrc = warm_pool.tile([P, 256], bf16, name="wsrc")
            nc.vector.memset(wsrc[:], 0.0)
            wps = ps1_pool.tile([P, 512], fp32, name="psg")
            for wi in range(N_WARMUP):
                nc.tensor.matmul(
                    wps[:, :256],
                    wsrc[:, :P],
                    wsrc[:],
                    start=(wi == 0),
                    stop=(wi == N_WARMUP - 1),
                )

            # prefetch state: (wgt/wut tiles by ft, xg, sct) per group
            pre = {}

            def issue_group_loads(gi):
                g0, gc = groups[gi]
                gsub = gc // P
                npre = min(N_PRE_FT, FT)
                # critical-path order: first f-tile's weights (sync queue)
                # race the token slab (gpsimd queue); remaining prefetched
                # f-tiles follow as one block DMA
                block = wgu_pre_pool.tile([P, npre, 2, DT, P], bf16, name="wgup")
                # gate half first: it alone gates the very first matmul chain
                nc.sync.dma_start(out=block[:, 0, 0], in_=wgu_ap[0][:, 0])
                nc.sync.dma_start(out=block[:, 0, 1], in_=wgu_ap[0][:, 1])
                # one tile per chunk, each filled by one fully-contiguous
                # DMA (host lays the slab out chunk-major): the first matmul
                # chain starts when just the first chunk has landed
                xg = []
                off = DT * g0
                for ci, cw in enumerate(_chunks(gc)):
                    xt = xg_pool.tile([P, DT, cw], bf16, name="xg")
                    if gi == 0 and ci == 0:
                        # dt-halves: the first matmul chain starts after half
                        # the chunk lands (early DMA is latency-bound)
                        h = DT // 2
                        nc.gpsimd.dma_start(
                            out=xt[:, :h, :],
                            in_=xT_ap[:, off : off + h * cw].rearrange(
                                "p (dt c) -> p dt c", c=cw
                            ),
                        )
                        nc.gpsimd.dma_start(
                            out=xt[:, h:, :],
                            in_=xT_ap[:, off + h * cw : off + DT * cw].rearrange(
                                "p (dt c) -> p dt c", c=cw
                            ),
                        )
                    else:
                        # later chunks ride the otherwise-idle scalar queue:
                        # three parallel supply lanes during the cold-DMA
                        # startup window (wd quarters have 100+us of slack)
                        nc.scalar.dma_start(
                            out=xt[:],
                            in_=xT_ap[:, off : off + DT * cw].rearrange(
                                "p (dt c) -> p dt c", c=cw
                            ),
                        )
                    xg.append(xt)
                    off += DT * cw
                sct = sc_pool.tile([P, GS], fp32, name="sct")
                nc.scalar.dma_start(
                    out=sct[:, :gsub], in_=sc_ap[:, g0 // P : g0 // P + gsub]
                )
                for ft in range(1, npre):
                    nc.sync.dma_start(out=block[:, ft], in_=wgu_ap[ft])
                wgu = {ft: block[:, ft] for ft in range(npre)}
                pre[gi] = (wgu, xg, sct)

            issue_group_loads(0)
            wd_res = wd_pool.tile([P, FT, D], bf16, name="wdr")

            for gi, (g0, gc) in enumerate(groups):
                gsub = gc // P
                batches = batch_plan(gsub, False)
                batches_last_d0 = batch_plan(gsub, gi == n_groups - 1)

                wgu_pre, xg, sct = pre.pop(gi)

                # equal-width chunks: keep every chunk >=320 so the fixed
                # per-matmul cost stays hidden; chunk ci lives in tile xg[ci]
                chunks = []
                c0 = 0
                for ci, cw in enumerate(_chunks(gc)):
                    chunks.append((ci, c0, cw))
                    c0 += cw

                # ---- stage 1: hT[f, c] = silu(wg.T x) * (wu.T x) ----
                ht_tiles = []
                for ft in range(FT):
                    if ft in wgu_pre:
                        wgut = wgu_pre.pop(ft)
                    else:
                        wgut = wgu_pool.tile([P, 2, DT, P], bf16, name="wgut")
                        nc.sync.dma_start(out=wgut[:], in_=wgu_ap[ft])
                    if gi == 0 and 1 <= ft <= 4:
                        # wd is small in bf16 (8.4 MB): stream it once into
                        # SBUF on the otherwise-idle scalar queue (in
                        # quarters, so stage 2's first f-tiles don't wait on
                        # the whole transfer); never touches HBM again
                        q8 = FT // 4
                        f0 = (ft - 1) * q8
                        nc.scalar.dma_start(
                            out=wd_res[:, f0 : f0 + q8, :],
                            in_=wd_ap[:, f0 : f0 + q8, :],
                        )
                    if ft == (24 if gc >= 640 else 12) and gi + 1 < n_groups:
                        # queue the next group's token slab + first weights
                        # behind the remaining stage-1 loads: they land
                        # during this group's stage 2
                        issue_group_loads(gi + 1)
                    ht = ht_pool.tile([P, G_MAX], bf16, name="ht")
                    ht_tiles.append(ht)
                    for ci, c0, cw in chunks:
                        psg = ps1_pool.tile([P, 512], fp32, name="psg")
                        for dt_i in range(DT):
                            nc.tensor.matmul(
                                psg[:, :cw],
                                wgut[:, 0, dt_i, :],
                                xg[ci][:, dt_i, :],
                                start=(dt_i == 0),
                                stop=(dt_i == DT - 1),
                            )
                        psu = ps1_pool.tile([P, 512], fp32, name="psu")
                        for dt_i in range(DT):
                            nc.tensor.matmul(
                                psu[:, :cw],
                                wgut[:, 1, dt_i, :],
                                xg[ci][:, dt_i, :],
                                start=(dt_i == 0),
                                stop=(dt_i == DT - 1),
                            )
                        sil = act_pool.tile([P, 512], fp32, name="sil")
                        nc.scalar.activation(sil[:, :cw], psg[:, :cw], silu_fn)
                        nc.vector.tensor_tensor(
                            out=ht[:, c0 : c0 + cw],
                            in0=sil[:, :cw],
                            in1=psu[:, :cw],
                            op=mult_op,
                        )

                # ---- stage 2: y[c, d] = sum_f hT[f, c] * wd[f, d], scaled ----
                for d0 in range(0, D_MODEL, 512):
                    cur_batches = batches_last_d0 if d0 == 512 else batches
                    for bi, cs_list in enumerate(cur_batches):
                        ps_out = [
                            ps2_pool.tile([P, 512], fp32, name="pso") for _ in cs_list
                        ]
                        for ft in range(FT):
                            for i, cs in enumerate(cs_list):
                                nc.tensor.matmul(
                                    ps_out[i][:],
                                    ht_tiles[ft][:, cs * P : (cs + 1) * P],
                                    wd_res[:, ft, d0 : d0 + 512],
                                    start=(ft == 0),
                                    stop=(ft == FT - 1),
                                )
                        for i, cs in enumerate(cs_list):
                            ot = out_pool.tile([P, 512], bf16, name="ot")
                            if i % 2 == 0:
                                nc.vector.tensor_scalar_mul(
                                    ot[:], ps_out[i][:], sct[:, cs : cs + 1]
                                )
                            else:
                                # spread evictions across engines so the bank
                                # ring frees faster at d0 boundaries
                                nc.scalar.activation(
                                    ot[:],
                                    ps_out[i][:],
                                    mybir.ActivationFunctionType.Copy,
                                    scale=sct[:, cs : cs + 1],
                                )
                            r0 = g0 + cs * P
                            nc.gpsimd.dma_start(
                                out=y_ap[r0 : r0 + P, d0 : d0 + 512], in_=ot[:]
                            )
    nc.compile()
    return nc


def _get_program(C):
    if C not in _programs:
        _programs[C] = _build_program(C)
    return _programs[C]


def _route(xf, router_w):
    """Host router, float64 (all f32 evaluation orders agree on this input's
    top-2 sets; f64 is the stable reference ranking). Mirrors
    softmax -> top_k(2) -> renormalize from the reference."""
    logits = xf.astype(np.float64) @ router_w.astype(np.float64).T
    logits -= logits.max(axis=-1, keepdims=True)
    sm = np.exp(logits)
    sm /= sm.sum(axis=-1, keepdims=True)
    top = np.argsort(-sm, axis=-1, kind="stable")[:, :TOP_K]
    tsc = np.take_along_axis(sm, top, axis=1)
    tsc = tsc / tsc.sum(axis=-1, keepdims=True)
    return top, tsc


def kernel(x, router_w, w_gate, w_up, w_down):
    global LAST_EXEC_NS
    from concourse.bass_utils import run_bass_kernel_spmd

    trace = os.environ.get("MOE_TRACE", "0") == "1"
    if trace:
        _ensure_axon_hooks()

    x = np.asarray(x, dtype=np.float32)
    router_w = np.asarray(router_w, dtype=np.float32)
    w_gate = np.asarray(w_gate, dtype=np.float32)
    w_up = np.asarray(w_up, dtype=np.float32)
    w_down = np.asarray(w_down, dtype=np.float32)

    B, T, D = x.shape
    N = B * T
    F = D_FF
    FT, DT = F // P, D // P
    xf = np.ascontiguousarray(x.reshape(N, D))

    top, tsc = _route(xf, router_w)

    tok_rows = []
    tok_wts = []
    for e in range(N_EXPERTS):
        mask = top == e
        rows = np.nonzero(mask.any(axis=1))[0]
        wts = tsc[mask].astype(np.float32)
        tok_rows.append(rows)
        tok_wts.append(wts)

    cmax = max(max(len(r) for r in tok_rows), 1)
    C = max(((cmax + P - 1) // P) * P, 256)

    nc = _get_program(C)

    # pre-tile weights host-side (bf16) so device DMAs are contiguous and
    # gate+up arrive in one DMA per f-tile:
    #   wguT[e][ft, p, 0/1, dt, j] = w_gate/w_up[e, dt*128+p, ft*128+j]
    #   wdT[e][p, ft, d]           = w_down[e, ft*128+p, d]
    wg16 = w_gate.astype(BF16).reshape(N_EXPERTS, DT, P, FT, P).transpose(0, 3, 2, 1, 4)
    wu16 = w_up.astype(BF16).reshape(N_EXPERTS, DT, P, FT, P).transpose(0, 3, 2, 1, 4)
    wgu16 = np.ascontiguousarray(np.stack([wg16, wu16], axis=3))
    wd16 = np.ascontiguousarray(
        w_down.astype(BF16).reshape(N_EXPERTS, FT, P, D).transpose(0, 2, 1, 3)
    )

    sizes = _group_sizes(C)
    in_maps = []
    for e in range(N_EXPERTS):
        rows = tok_rows[e]
        xg = np.zeros((C, D), np.float32)
        xg[: len(rows)] = xf[rows]
        # group-major flat layout: per group a contiguous [dt, c] slab per
        # partition row
        x16 = xg.astype(BF16).reshape(C, DT, P)
        parts = []
        g0 = 0
        for gc in sizes:
            for cw in _chunks(gc):
                parts.append(
                    x16[g0 : g0 + cw].transpose(2, 1, 0).reshape(P, DT * cw)
                )
                g0 += cw
        xflat = np.ascontiguousarray(np.concatenate(parts, axis=1))
        scv = np.zeros((C,), np.float32)
        scv[: len(rows)] = tok_wts[e]
        in_maps.append(
            {
                "xT": xflat,
                "wgu": wgu16[e],
                "wd": wd16[e],
                "sc": np.ascontiguousarray(scv.reshape(C // P, P).T),
            }
        )

    res = run_bass_kernel_spmd(nc, in_maps, list(range(N_EXPERTS)), trace=trace)
    if trace:
        LAST_EXEC_NS = res.exec_time_ns

    out = np.zeros((N, D), np.float32)
    for e in range(N_EXPERTS):
        rows = tok_rows[e]
        out[rows] += res.results[e]["y"][: len(rows)].astype(np.float32)
    return out.reshape(B, T, D)
